# revision 41
# baseline (speedup 1.0000x reference)
"""Trainium2 Bass kernel for LocalSquaredDistanceLayer (shapelet min-distance).

Math (matching the reference exactly):
  x_norm   = z-normalize x over time per (batch, channel)
  kern     = z-normalize kernel per shapelet over (KSZ, C)
  For output element out[b, t, kp] with kp = 4*ch + jo (ch = kp//4, jo = kp%4):
     w = x_norm[b, t+8jo : t+8jo+8, ch]               (8 consecutive samples)
     out[b,t,kp] = min_s || w - kern[s, kp, :] ||^2

Device algorithm per core (2 batches per core, kernel replicated):
  - x loaded contiguously, PE-transposed to Xsig[c, b*T+t]; z-normalized
    per batch; staged to DRAM per batch
  - Hall[65, 16*512] f32r: rows 0-31 shifted x (Hankel via flat DRAM reads),
    rows 32-63 shifted x^2, row 64 ones
  - Fall[65, 8*256] f32r, col = jo*512 + ch*64 + s: block-diagonal taps
    (-2*kern^T via PE transposes), ones blocks, K2 row
  - per (b, tchunk): 2 psum half-groups of 4 fp32r matmuls each
    (psum [128, 1024], 4 banks in flight) = squared distances for
    32 kp-groups x 64 shapelets
  - min over shapelets on Vector per half-group; stores on Sync/Scalar
"""

import sys

for _p in ("/opt/trn_rl_repo",):
    if _p not in sys.path:
        sys.path.insert(0, _p)

import numpy as np

B, T, C = 16, 512, 8
S, KSZ = 64, 32
TOUT = T - KSZ + 1  # 481
NCORES = 8
BPC = B // NCORES  # batches per core
NSIG = BPC * C  # signals per core
EPS = 1e-8
XPAD = 544  # padded signal length (hankel reads up to 511+31)

_cache = {}


def _rap(base, dims, extra=0):
    """Raw AP at base slice's offset (+extra elems) with [step, count] dims."""
    import concourse.bass as bass

    return bass.AP(tensor=base.tensor, offset=base.offset + extra,
                   ap=[list(d) for d in dims])


def _build_nc():
    import concourse.bass as bass
    import concourse.bacc as bacc
    import concourse.tile as tile
    from concourse import mybir
    from concourse.masks import make_identity
    from contextlib import ExitStack

    f32 = mybir.dt.float32
    f32r = mybir.dt.float32r
    bf16 = mybir.dt.bfloat16
    AX = mybir.AxisListType
    OP = mybir.AluOpType
    ACT = mybir.ActivationFunctionType

    nc = bacc.Bacc("TRN2", target_bir_lowering=False, debug=False)
    x_d = nc.dram_tensor("x", [BPC, T, C], f32, kind="ExternalInput").ap()
    k_d = nc.dram_tensor("kernel", [S, KSZ, C], f32, kind="ExternalInput").ap()
    o_d = nc.dram_tensor("out", [BPC, TOUT, KSZ], f32, kind="ExternalOutput").ap()

    with tile.TileContext(nc) as tc, ExitStack() as ctx:
        const = ctx.enter_context(tc.tile_pool(name="const", bufs=1))
        outp = ctx.enter_context(tc.tile_pool(name="outp", bufs=4))
        dram = ctx.enter_context(tc.tile_pool(name="dram", bufs=1, space="DRAM"))

        Hall = const.tile([65, NSIG * T], f32, tag="Hall")
        Fall = const.tile([65, C * 4 * S], f32, tag="Fall")
        ones8k = const.tile([16, 512], f32, tag="ones8k")
        Xsig = const.tile([C, BPC * T], f32, tag="Xsig")
        Xn = const.tile([C, BPC * XPAD], f32, tag="Xn")

        with tc.tile_pool(name="ldp", bufs=1) as ldp, \
             tc.tile_pool(name="pprep", bufs=1, space="PSUM") as pprep, \
             tc.tile_pool(name="pxp", bufs=2, space="PSUM") as pxp:
            identS = ldp.tile([128, 128], f32, tag="ident")
            make_identity(nc, identS[:])

            # ---- early independent work ----
            KN = ldp.tile([S, KSZ * C], f32, tag="KN")
            nc.gpsimd.dma_start(out=KN[:], in_=k_d.rearrange("s k c -> s (k c)"))
            nc.gpsimd.memset(ones8k[:], 1.0)
            nc.gpsimd.dma_start(out=Hall[64:65, :].bitcast(f32r),
                                in_=ones8k[:].bitcast(f32r))
            nc.gpsimd.memset(Fall[0:64, :], 0.0)
            for j in range(4):
                # ones blocks for the x^2 window sum
                nc.gpsimd.dma_start(
                    out=Fall[32 + 8 * j:40 + 8 * j, 512 * j:512 * (j + 1)].bitcast(f32r),
                    in_=ones8k[0:C, :].bitcast(f32r))
            nc.vector.memset(Xn[:], 0.0)

            # ---- x load (contiguous) + PE transpose + strided copies ----
            # X0[p, q] = x_flat[b, p*32 + q]  (t = 4p + q//8, c = q%8)
            for b in range(BPC):
                X0 = ldp.tile([128, 32], f32, tag=f"X0_{b}", name=f"X0_{b}")
                nc.sync.dma_start(
                    out=X0[:],
                    in_=_rap(x_d[b, 0:1, 0:1], [[32, 128], [1, 32]]))
                for ts in range(4):
                    PX = pxp.tile([8, 128], f32, tag="PX")
                    nc.tensor.transpose(
                        PX[:], X0[:, 8 * ts:8 * ts + 8], identS[:])
                    # PX[c, p] = x[b, 4p + ts, c] -> Xsig[c, b*T + 4p + ts]
                    nc.vector.tensor_copy(
                        out=_rap(Xsig[0:1, b * T + ts:b * T + ts + 1],
                                 [[BPC * T, C], [4, 128]]),
                        in_=PX[:])

            # ---- kernel prep chain (F needed only by ~10us) ----
            kst = ldp.tile([S, nc.vector.BN_STATS_DIM], f32, tag="kst")
            nc.vector.bn_stats(out=kst[:], in_=KN[:])
            mvk = ldp.tile([S, nc.vector.BN_AGGR_DIM], f32, tag="mvk")
            nc.vector.bn_aggr(out=mvk[:], in_=kst[:])
            XnD = dram.tile([C, BPC * XPAD], f32, tag="XnD")

            def _xchain(b):
                xst = ldp.tile([C, nc.vector.BN_STATS_DIM], f32,
                               tag=f"xst{b}", name=f"xst{b}")
                nc.vector.bn_stats(out=xst[:], in_=Xsig[:, b * T:(b + 1) * T])
                mvx = ldp.tile([C, nc.vector.BN_AGGR_DIM], f32,
                               tag=f"mvx{b}", name=f"mvx{b}")
                nc.vector.bn_aggr(out=mvx[:], in_=xst[:])
                xstd = ldp.tile([C, 1], f32, tag=f"xstd{b}", name=f"xstd{b}")
                nc.scalar.activation(out=xstd[:], in_=mvx[:, 1:2], func=ACT.Sqrt)
                nc.vector.tensor_scalar_add(xstd[:], xstd[:], EPS)
                xrstd = ldp.tile([C, 1], f32, tag=f"xrstd{b}", name=f"xrstd{b}")
                nc.vector.reciprocal(out=xrstd[:], in_=xstd[:])
                xbias = ldp.tile([C, 1], f32, tag=f"xbias{b}", name=f"xbias{b}")
                nc.vector.scalar_tensor_tensor(
                    out=xbias[:], in0=mvx[:, 0:1], scalar=-1.0, in1=xrstd[:],
                    op0=OP.mult, op1=OP.mult)
                nc.vector.tensor_scalar(
                    out=Xn[:, b * XPAD:b * XPAD + T].bitcast(f32r),
                    in0=Xsig[:, b * T:(b + 1) * T], scalar1=xrstd[:],
                    scalar2=xbias[:], op0=OP.mult, op1=OP.add)
                nc.sync.dma_start(
                    out=XnD[:, b * XPAD:(b + 1) * XPAD].bitcast(f32r),
                    in_=Xn[:, b * XPAD:(b + 1) * XPAD].bitcast(f32r))

            _xchain(0)
            # ---- kernel norm chain (fills b0->b1 gap) ----
            kstd = ldp.tile([S, 1], f32, tag="kstd")
            nc.scalar.activation(out=kstd[:], in_=mvk[:, 1:2], func=ACT.Sqrt)
            nc.vector.tensor_scalar_add(kstd[:], kstd[:], EPS)
            krstd = ldp.tile([S, 1], f32, tag="krstd")
            nc.vector.reciprocal(out=krstd[:], in_=kstd[:])
            kscale = ldp.tile([S, 1], f32, tag="kscale")
            nc.vector.tensor_scalar_mul(kscale[:], krstd[:], -2.0)
            kbias = ldp.tile([S, 1], f32, tag="kbias")
            nc.vector.scalar_tensor_tensor(
                out=kbias[:], in0=mvk[:, 0:1], scalar=2.0, in1=krstd[:],
                op0=OP.mult, op1=OP.mult)
            KNm = ldp.tile([S, KSZ * C], f32, tag="KNm")
            nc.vector.tensor_scalar(
                out=KNm[:], in0=KN[:], scalar1=kscale[:], scalar2=kbias[:],
                op0=OP.mult, op1=OP.add)
            KN2 = ldp.tile([S, KSZ * C], f32, tag="KN2")
            nc.vector.tensor_tensor(out=KN2[:], in0=KNm[:], in1=KNm[:],
                                    op=OP.mult)
            K2sn = ldp.tile([S, KSZ], f32, tag="K2sn")
            nc.vector.tensor_reduce(
                out=K2sn[:],
                in_=KN2[:].rearrange("s (ch j c) -> s j ch c", ch=C, j=4, c=C),
                axis=AX.X, op=OP.add)
            nc.vector.tensor_scalar_mul(K2sn[:], K2sn[:], 0.25)

            # ---- F staging via PE transposes ----
            TP = pprep.tile([C, 4 * C * S], f32, tag="TP")
            for kp in range(KSZ):
                ch, jo = kp // 4, kp % 4
                nc.tensor.transpose(
                    TP[:, jo * 512 + ch * S:jo * 512 + ch * S + S],
                    KNm[:, kp * C:(kp + 1) * C],
                    identS[0:S, 0:S])
            K2T = pprep.tile([KSZ, S], f32, tag="K2T")
            nc.tensor.transpose(K2T[:], K2sn[:], identS[0:S, 0:S])
            Fx = ldp.tile([C, 4 * C * S], f32, tag="Fx")
            nc.scalar.copy(out=Fx[:].bitcast(f32r), in_=TP[:])
            K2sb = ldp.tile([KSZ, S], f32, tag="K2sb")
            nc.scalar.copy(out=K2sb[:].bitcast(f32r), in_=K2T[:])

            # ---- F scatter: all-2D SBUF->SBUF block DMAs ----
            for j in range(4):
                nc.gpsimd.dma_start(
                    out=Fall[8 * j:8 * j + 8, 512 * j:512 * (j + 1)].bitcast(f32r),
                    in_=Fx[:, 512 * j:512 * (j + 1)].bitcast(f32r))
                nc.gpsimd.dma_start(
                    out=Fall[64:65, 512 * j:512 * (j + 1)].bitcast(f32r),
                    in_=K2sb[C * j:C * (j + 1), :].bitcast(f32r))
            _xchain(1)

            # ---- Hankel x rows via flat DRAM reads; squares in SBUF ----
            dma_engines = [nc.sync, nc.scalar]
            for sig in range(NSIG):
                b, ch = sig // C, sig % C
                off = ch * (BPC * XPAD) + b * XPAD
                eng = dma_engines[sig % 2]
                eng.dma_start(
                    out=Hall[0:KSZ, sig * T:(sig + 1) * T].bitcast(f32r),
                    in_=_rap(XnD[0:1, 0:1],
                             [[1, KSZ], [1, T]], extra=off).bitcast(f32r))
                sl = Hall[KSZ:2 * KSZ, sig * T:(sig + 1) * T].bitcast(f32r)
                s0 = Hall[0:KSZ, sig * T:(sig + 1) * T]
                if sig % 3 == 0:
                    nc.scalar.activation(out=sl, in_=s0, func=ACT.Square)
                elif sig % 3 == 1:
                    nc.vector.tensor_tensor(out=sl, in0=s0, in1=s0, op=OP.mult)
                else:
                    nc.gpsimd.tensor_tensor(out=sl, in0=s0, in1=s0, op=OP.mult)

        # ---- main: fp32r matmuls + min-reduce + store ----
        with tc.tile_pool(name="pmm", bufs=4, space="PSUM") as pmm:
            for b in range(BPC):
                for cc in range(4):
                    gi = b * 4 + cc
                    c0 = cc * 128
                    cnt = 128 if cc < 3 else TOUT - 3 * 128
                    PM = outp.tile([128, KSZ], f32, tag="PM")
                    for h in range(2):
                        acc = pmm.tile([128, C * 2 * S], f32, tag="acc")
                        for chh in range(C // 2):
                            ch = h * 4 + chh
                            sig = b * C + ch
                            rhs = _rap(Fall[0:65, ch * S:ch * S + 1],
                                       [[2048, 65], [512, 4], [1, S]])
                            nc.tensor.matmul(
                                acc[:, chh * 256:(chh + 1) * 256],
                                lhsT=Hall[0:65, sig * T + c0:sig * T + c0 + 128].bitcast(f32r),
                                rhs=rhs.bitcast(f32r),
                                start=True, stop=True)
                        nc.vector.tensor_reduce(
                            out=PM[:, h * 16:(h + 1) * 16],
                            in_=acc[:].rearrange("p (g s) -> p g s", s=S),
                            axis=AX.X, op=OP.min)
                    st = nc.sync if gi % 2 == 0 else nc.scalar
                    st.dma_start(
                        out=o_d[b, c0:c0 + cnt, :], in_=PM[0:cnt, :])

    nc.compile()
    return nc


def get_nc():
    if "nc" not in _cache:
        _cache["nc"] = _build_nc()
    return _cache["nc"]


def kernel(x: np.ndarray, kernel: np.ndarray) -> np.ndarray:
    from concourse.bass_utils import run_bass_kernel_spmd

    nc = get_nc()
    x = np.ascontiguousarray(x, dtype=np.float32)
    kern = np.ascontiguousarray(kernel, dtype=np.float32)
    in_maps = [
        {"x": x[i * BPC:(i + 1) * BPC], "kernel": kern} for i in range(NCORES)
    ]
    res = run_bass_kernel_spmd(nc, in_maps, core_ids=list(range(NCORES)))
    return np.concatenate([r["out"] for r in res.results], axis=0)


if __name__ == "__main__":
    rng = np.random.default_rng(0)
    x = rng.standard_normal((B, T, C), dtype=np.float32)
    k = rng.uniform(-0.05, 0.05, (S, KSZ, C)).astype(np.float32)
    out = kernel(x=x, kernel=k)
    print(out.shape, out.dtype)
